# revision 2
# baseline (speedup 1.0000x reference)
"""Trainium2 Bass kernel for one DenseOSTL step (Dense + LIF w/ eligibility traces).

Reference semantics (all f32):
    h      = x @ W + b                  # [B, H]
    u      = beta * Vmem + h            # [B, H]
    spikes = (u > thresh).astype(f32)   # [B, H]
    Vmem'  = u - spikes
    E_W'   = beta * E_W + x[:, :, None] # [B, D, H]
    E_b'   = beta * E_b + 1             # [B, H]

Sharding: batch B=64 split across 8 NeuronCores (8 rows each); W/b replicated.
No collectives needed (forward only). The kernel is HBM-bound on the E_W
stream (32 MB in + 32 MB out per core).
"""

import numpy as np

import concourse.bass as bass
import concourse.mybir as mybir
import concourse.tile as tile
from concourse import bacc
from concourse.bass_utils import run_bass_kernel_spmd

F32 = mybir.dt.float32
ALU = mybir.AluOpType

B, D, H = 64, 1024, 1024
NCORES = 8
BS = B // NCORES            # 8 batch rows per core
P = 128                     # SBUF partitions
KCH = D // P                # 8 contraction chunks
ROWS_FLAT = BS * D          # 8192 flat (b, d) rows per core
R = 4                       # E_W rows folded into one partition's free dim
FREE = R * H                # 4096 f32 = 16 KB per partition per tile (2 MB tile)
NT = ROWS_FLAT // (P * R)   # 16 streaming tiles
EW_BUFS = 4

BETA = 0.95
THRESH = 1.0

_PROG = None  # (nc, core_ids) cache — compile once per process


def build_program(ew_passes: int = 1):
    """Emit + compile the SPMD single-core program (same program on all 8 cores).

    ew_passes > 1 repeats the E_W streaming phase (same data) for timing
    experiments; the graded kernel uses ew_passes=1.
    """
    nc = bacc.Bacc("TRN2", target_bir_lowering=False, debug=False,
                   num_devices=NCORES)

    w_d = nc.dram_tensor("W", [D, H], F32, kind="ExternalInput").ap()
    b_d = nc.dram_tensor("bvec", [1, H], F32, kind="ExternalInput").ap()
    xt_d = nc.dram_tensor("xT", [P, KCH * BS], F32, kind="ExternalInput").ap()
    xc_d = nc.dram_tensor("xcols", [R, P, NT], F32, kind="ExternalInput").ap()
    vm_d = nc.dram_tensor("Vmem", [BS, H], F32, kind="ExternalInput").ap()
    eb_d = nc.dram_tensor("E_b", [BS, H], F32, kind="ExternalInput").ap()
    ew_d = nc.dram_tensor("E_W", [ROWS_FLAT, H], F32, kind="ExternalInput").ap()

    vmo_d = nc.dram_tensor("Vmem_out", [BS, H], F32, kind="ExternalOutput").ap()
    spo_d = nc.dram_tensor("spikes_out", [BS, H], F32, kind="ExternalOutput").ap()
    ebo_d = nc.dram_tensor("E_b_out", [BS, H], F32, kind="ExternalOutput").ap()
    ewo_d = nc.dram_tensor("E_W_out", [ROWS_FLAT, H], F32, kind="ExternalOutput").ap()

    # DRAM views for streaming: partition p of tile t covers flat rows
    # (t*128+p)*R + j, j-th H-block living at free cols [j*H, (j+1)*H).
    ew_v = ew_d.rearrange("(t p j) h -> t p (j h)", p=P, j=R)
    ewo_v = ewo_d.rearrange("(t p j) h -> t p (j h)", p=P, j=R)

    with tile.TileContext(nc) as tc:
        with (
            tc.tile_pool(name="const", bufs=1) as const_pool,
            tc.tile_pool(name="small", bufs=1) as small_pool,
            tc.tile_pool(name="psum", bufs=2, space="PSUM") as psum_pool,
            tc.tile_pool(name="ew", bufs=EW_BUFS) as ew_pool,
        ):
            # --- constants / small operands ---
            w_sb = const_pool.tile([P, KCH * H], F32)
            nc.sync.dma_start(
                out=w_sb[:, :].rearrange("p (k h) -> p k h", k=KCH),
                in_=w_d.rearrange("(k p) h -> p k h", p=P),
            )
            xt_sb = const_pool.tile([P, KCH * BS], F32)
            nc.sync.dma_start(out=xt_sb[:, :], in_=xt_d)
            xc_sb = const_pool.tile([P, R * NT], F32)
            nc.sync.dma_start(
                out=xc_sb[:, :].rearrange("p (j i) -> p j i", j=R),
                in_=xc_d.rearrange("j p i -> p j i"),
            )
            b_sb = const_pool.tile([1, H], F32)
            nc.sync.dma_start(out=b_sb[:, :], in_=b_d)
            ones_sb = const_pool.tile([1, BS], F32)
            nc.vector.memset(ones_sb[:, :], 1.0)

            vm_sb = small_pool.tile([BS, H], F32)
            nc.sync.dma_start(out=vm_sb[:, :], in_=vm_d)
            eb_sb = small_pool.tile([BS, H], F32)
            nc.sync.dma_start(out=eb_sb[:, :], in_=eb_d)

            u_sb = small_pool.tile([BS, H], F32)
            sp_sb = small_pool.tile([BS, H], F32)
            vn_sb = small_pool.tile([BS, H], F32)
            ebn_sb = small_pool.tile([BS, H], F32)

            # --- h = x @ W + b on PE (fp32), two PSUM banks of N=512 ---
            NB = 512
            for n in range(H // NB):
                ps = psum_pool.tile([BS, NB], F32)
                for k in range(KCH):
                    nc.tensor.matmul(
                        ps[:, :],
                        xt_sb[:, k * BS:(k + 1) * BS],
                        w_sb[:, k * H + n * NB: k * H + n * NB + NB],
                        start=(k == 0),
                        stop=False,
                    )
                # bias via K=1 matmul: ones[1, BS].T @ b[1, NB]
                nc.tensor.matmul(
                    ps[:, :],
                    ones_sb[0:1, :],
                    b_sb[0:1, n * NB:(n + 1) * NB],
                    start=False,
                    stop=True,
                )
                # u = (Vmem * beta) + h
                nc.vector.scalar_tensor_tensor(
                    out=u_sb[:, n * NB:(n + 1) * NB],
                    in0=vm_sb[:, n * NB:(n + 1) * NB],
                    scalar=BETA,
                    in1=ps[:, :],
                    op0=ALU.mult,
                    op1=ALU.add,
                )

            nc.vector.tensor_scalar(
                out=sp_sb[:, :], in0=u_sb[:, :],
                scalar1=THRESH, scalar2=None, op0=ALU.is_gt,
            )
            nc.vector.tensor_tensor(
                out=vn_sb[:, :], in0=u_sb[:, :], in1=sp_sb[:, :], op=ALU.subtract,
            )
            nc.vector.tensor_scalar(
                out=ebn_sb[:, :], in0=eb_sb[:, :],
                scalar1=BETA, scalar2=1.0, op0=ALU.mult, op1=ALU.add,
            )
            nc.sync.dma_start(out=spo_d, in_=sp_sb[:, :])
            nc.sync.dma_start(out=vmo_d, in_=vn_sb[:, :])
            nc.sync.dma_start(out=ebo_d, in_=ebn_sb[:, :])

            # --- E_W' = beta * E_W + x, streamed in 2 MB tiles ---
            for _ in range(ew_passes):
                for t in range(NT):
                    tl = ew_pool.tile([P, FREE], F32, tag="ewt")
                    nc.sync.dma_start(out=tl[:, :], in_=ew_v[t])
                    for j in range(R):
                        nc.vector.tensor_scalar(
                            out=tl[:, j * H:(j + 1) * H],
                            in0=tl[:, j * H:(j + 1) * H],
                            scalar1=BETA,
                            scalar2=xc_sb[:, j * NT + t: j * NT + t + 1],
                            op0=ALU.mult,
                            op1=ALU.add,
                        )
                    nc.sync.dma_start(out=ewo_v[t], in_=tl[:, :])

    nc.compile()
    return nc


def make_in_maps(W, b, Vmem, E_W, E_b, x):
    """Per-core input dicts (numpy, f32, C-contiguous)."""
    W = np.ascontiguousarray(np.asarray(W, np.float32))
    b = np.asarray(b, np.float32).reshape(1, H)
    Vmem = np.asarray(Vmem, np.float32)
    E_W = np.asarray(E_W, np.float32)
    E_b = np.asarray(E_b, np.float32)
    x = np.asarray(x, np.float32)

    in_maps = []
    for c in range(NCORES):
        sl = slice(c * BS, (c + 1) * BS)
        x_s = x[sl]                                   # [BS, D]
        # xT[p, k*BS + j] = x_s[j, k*128 + p]  (lhsT chunks for the matmul)
        xT = np.ascontiguousarray(
            x_s.reshape(BS, KCH, P).transpose(2, 1, 0).reshape(P, KCH * BS))
        # xcols[j, p, t] = x_flat[(t*128+p)*R + j]  (per-partition adds for E_W)
        x_flat = x_s.reshape(-1)
        xcols = np.ascontiguousarray(
            x_flat.reshape(NT, P, R).transpose(2, 1, 0))
        in_maps.append({
            "W": W,
            "bvec": b,
            "xT": xT,
            "xcols": xcols,
            "Vmem": np.ascontiguousarray(Vmem[sl]),
            "E_b": np.ascontiguousarray(E_b[sl]),
            "E_W": np.ascontiguousarray(E_W[sl].reshape(ROWS_FLAT, H)),
        })
    return in_maps


def assemble_outputs(results):
    """results: per-core dicts -> full (Vmem', E_W', E_b', spikes) tuple."""
    vm = np.concatenate([r["Vmem_out"] for r in results], axis=0)
    ew = np.concatenate(
        [r["E_W_out"].reshape(BS, D, H) for r in results], axis=0)
    eb = np.concatenate([r["E_b_out"] for r in results], axis=0)
    sp = np.concatenate([r["spikes_out"] for r in results], axis=0)
    return vm, ew, eb, sp


def kernel(W, b, Vmem, E_W, E_b, x):
    global _PROG
    if _PROG is None:
        _PROG = build_program()
    nc = _PROG
    in_maps = make_in_maps(W, b, Vmem, E_W, E_b, x)
    res = run_bass_kernel_spmd(nc, in_maps, list(range(NCORES))).results
    return assemble_outputs(res)


# revision 4
# speedup vs baseline: 1.0770x; 1.0770x over previous
"""Trainium2 Bass kernel for one DenseOSTL step (Dense + LIF w/ eligibility traces).

Reference semantics (all f32):
    h      = x @ W + b                  # [B, H]
    u      = beta * Vmem + h            # [B, H]
    spikes = (u > thresh).astype(f32)   # [B, H]
    Vmem'  = u - spikes
    E_W'   = beta * E_W + x[:, :, None] # [B, D, H]
    E_b'   = beta * E_b + 1             # [B, H]

Sharding: batch B=64 split across 8 NeuronCores (8 rows each); W/b replicated.
No collectives needed (forward only). The kernel is HBM-bound on the E_W
stream (32 MB in + 32 MB out per core).
"""

import numpy as np

import concourse.bass as bass
import concourse.mybir as mybir
import concourse.tile as tile
from concourse import bacc
from concourse.bass_utils import run_bass_kernel_spmd

F32 = mybir.dt.float32
ALU = mybir.AluOpType

B, D, H = 64, 1024, 1024
NCORES = 8
BS = B // NCORES            # 8 batch rows per core
P = 128                     # SBUF partitions
KCH = D // P                # 8 contraction chunks
ROWS_FLAT = BS * D          # 8192 flat (b, d) rows per core
R = 4                       # E_W rows folded into one partition's free dim
FREE = R * H                # 4096 f32 = 16 KB per partition per tile (2 MB tile)
NT = ROWS_FLAT // (P * R)   # 16 streaming tiles
EW_BUFS = 6

BETA = 0.95
THRESH = 1.0

_PROG = None  # (nc, core_ids) cache — compile once per process


def build_program(ew_passes: int = 1):
    """Emit + compile the SPMD single-core program (same program on all 8 cores).

    ew_passes > 1 repeats the E_W streaming phase (same data) for timing
    experiments; the graded kernel uses ew_passes=1.
    """
    nc = bacc.Bacc("TRN2", target_bir_lowering=False, debug=False,
                   num_devices=NCORES)

    w_d = nc.dram_tensor("W", [D, H], F32, kind="ExternalInput").ap()
    b_d = nc.dram_tensor("bvec", [1, H], F32, kind="ExternalInput").ap()
    xt_d = nc.dram_tensor("xT", [P, KCH * BS], F32, kind="ExternalInput").ap()
    xc_d = nc.dram_tensor("xcols", [R, P, NT], F32, kind="ExternalInput").ap()
    vm_d = nc.dram_tensor("Vmem", [BS, H], F32, kind="ExternalInput").ap()
    eb_d = nc.dram_tensor("E_b", [BS, H], F32, kind="ExternalInput").ap()
    ew_d = nc.dram_tensor("E_W", [ROWS_FLAT, H], F32, kind="ExternalInput").ap()

    vmo_d = nc.dram_tensor("Vmem_out", [BS, H], F32, kind="ExternalOutput").ap()
    spo_d = nc.dram_tensor("spikes_out", [BS, H], F32, kind="ExternalOutput").ap()
    ebo_d = nc.dram_tensor("E_b_out", [BS, H], F32, kind="ExternalOutput").ap()
    ewo_d = nc.dram_tensor("E_W_out", [ROWS_FLAT, H], F32, kind="ExternalOutput").ap()

    # DRAM views for streaming: partition p of tile t covers flat rows
    # (t*128+p)*R + j, j-th H-block living at free cols [j*H, (j+1)*H).
    ew_v = ew_d.rearrange("(t p j) h -> t p (j h)", p=P, j=R)
    ewo_v = ewo_d.rearrange("(t p j) h -> t p (j h)", p=P, j=R)

    with tile.TileContext(nc) as tc:
        with (
            tc.tile_pool(name="const", bufs=1) as const_pool,
            tc.tile_pool(name="small", bufs=1) as small_pool,
            tc.tile_pool(name="psum", bufs=2, space="PSUM") as psum_pool,
            tc.tile_pool(name="ew", bufs=EW_BUFS) as ew_pool,
        ):
            # In-DMAs issue from nc.sync (SP HWDGE ring); out-DMAs from
            # nc.scalar (ACT HWDGE ring) so the pre-store waits on DVE
            # completion never block input prefetch issue.

            # xcols first — first E_W tile's compute depends on it.
            xc_sb = const_pool.tile([P, R * NT], F32)
            nc.sync.dma_start(
                out=xc_sb[:, :].rearrange("p (j i) -> p j i", j=R),
                in_=xc_d.rearrange("j p i -> p j i"),
            )

            def ew_tile(t):
                tl = ew_pool.tile([P, FREE], F32, tag="ewt")
                nc.sync.dma_start(out=tl[:, :], in_=ew_v[t])
                for j in range(R):
                    nc.vector.tensor_scalar(
                        out=tl[:, j * H:(j + 1) * H],
                        in0=tl[:, j * H:(j + 1) * H],
                        scalar1=BETA,
                        scalar2=xc_sb[:, j * NT + t: j * NT + t + 1],
                        op0=ALU.mult,
                        op1=ALU.add,
                    )
                nc.scalar.dma_start(out=ewo_v[t], in_=tl[:, :])

            # Kick the E_W stream before the (latency-tolerant) matmul phase.
            head = 2
            for t in range(head):
                ew_tile(t)

            # --- constants / small operands ---
            w_sb = const_pool.tile([P, KCH * H], F32)
            w_v = w_d.rearrange("(k p) h -> k p h", p=P)
            for k in range(KCH):
                nc.sync.dma_start(
                    out=w_sb[:, k * H:(k + 1) * H], in_=w_v[k])
            xt_sb = const_pool.tile([P, KCH * BS], F32)
            nc.sync.dma_start(out=xt_sb[:, :], in_=xt_d)
            b_sb = const_pool.tile([1, H], F32)
            nc.sync.dma_start(out=b_sb[:, :], in_=b_d)
            ones_sb = const_pool.tile([1, BS], F32)
            nc.vector.memset(ones_sb[:, :], 1.0)

            vm_sb = small_pool.tile([BS, H], F32)
            nc.sync.dma_start(out=vm_sb[:, :], in_=vm_d)
            eb_sb = small_pool.tile([BS, H], F32)
            nc.sync.dma_start(out=eb_sb[:, :], in_=eb_d)

            u_sb = small_pool.tile([BS, H], F32)
            sp_sb = small_pool.tile([BS, H], F32)
            vn_sb = small_pool.tile([BS, H], F32)
            ebn_sb = small_pool.tile([BS, H], F32)

            # --- h = x @ W + b on PE (fp32), two PSUM banks of N=512 ---
            NB = 512
            for n in range(H // NB):
                ps = psum_pool.tile([BS, NB], F32)
                for k in range(KCH):
                    nc.tensor.matmul(
                        ps[:, :],
                        xt_sb[:, k * BS:(k + 1) * BS],
                        w_sb[:, k * H + n * NB: k * H + n * NB + NB],
                        start=(k == 0),
                        stop=False,
                    )
                # bias via K=1 matmul: ones[1, BS].T @ b[1, NB]
                nc.tensor.matmul(
                    ps[:, :],
                    ones_sb[0:1, :],
                    b_sb[0:1, n * NB:(n + 1) * NB],
                    start=False,
                    stop=True,
                )
                # u = (Vmem * beta) + h
                nc.vector.scalar_tensor_tensor(
                    out=u_sb[:, n * NB:(n + 1) * NB],
                    in0=vm_sb[:, n * NB:(n + 1) * NB],
                    scalar=BETA,
                    in1=ps[:, :],
                    op0=ALU.mult,
                    op1=ALU.add,
                )

            nc.vector.tensor_scalar(
                out=sp_sb[:, :], in0=u_sb[:, :],
                scalar1=THRESH, scalar2=None, op0=ALU.is_gt,
            )
            nc.vector.tensor_tensor(
                out=vn_sb[:, :], in0=u_sb[:, :], in1=sp_sb[:, :], op=ALU.subtract,
            )
            nc.vector.tensor_scalar(
                out=ebn_sb[:, :], in0=eb_sb[:, :],
                scalar1=BETA, scalar2=1.0, op0=ALU.mult, op1=ALU.add,
            )
            nc.scalar.dma_start(out=spo_d, in_=sp_sb[:, :])
            nc.scalar.dma_start(out=vmo_d, in_=vn_sb[:, :])
            nc.scalar.dma_start(out=ebo_d, in_=ebn_sb[:, :])

            # --- E_W' = beta * E_W + x, streamed in 2 MB tiles ---
            for t in range(head, NT):
                ew_tile(t)
            for _ in range(ew_passes - 1):
                for t in range(NT):
                    ew_tile(t)

    nc.compile()
    return nc


def make_in_maps(W, b, Vmem, E_W, E_b, x):
    """Per-core input dicts (numpy, f32, C-contiguous)."""
    W = np.ascontiguousarray(np.asarray(W, np.float32))
    b = np.asarray(b, np.float32).reshape(1, H)
    Vmem = np.asarray(Vmem, np.float32)
    E_W = np.asarray(E_W, np.float32)
    E_b = np.asarray(E_b, np.float32)
    x = np.asarray(x, np.float32)

    in_maps = []
    for c in range(NCORES):
        sl = slice(c * BS, (c + 1) * BS)
        x_s = x[sl]                                   # [BS, D]
        # xT[p, k*BS + j] = x_s[j, k*128 + p]  (lhsT chunks for the matmul)
        xT = np.ascontiguousarray(
            x_s.reshape(BS, KCH, P).transpose(2, 1, 0).reshape(P, KCH * BS))
        # xcols[j, p, t] = x_flat[(t*128+p)*R + j]  (per-partition adds for E_W)
        x_flat = x_s.reshape(-1)
        xcols = np.ascontiguousarray(
            x_flat.reshape(NT, P, R).transpose(2, 1, 0))
        in_maps.append({
            "W": W,
            "bvec": b,
            "xT": xT,
            "xcols": xcols,
            "Vmem": np.ascontiguousarray(Vmem[sl]),
            "E_b": np.ascontiguousarray(E_b[sl]),
            "E_W": np.ascontiguousarray(E_W[sl].reshape(ROWS_FLAT, H)),
        })
    return in_maps


def assemble_outputs(results):
    """results: per-core dicts -> full (Vmem', E_W', E_b', spikes) tuple."""
    vm = np.concatenate([r["Vmem_out"] for r in results], axis=0)
    ew = np.concatenate(
        [r["E_W_out"].reshape(BS, D, H) for r in results], axis=0)
    eb = np.concatenate([r["E_b_out"] for r in results], axis=0)
    sp = np.concatenate([r["spikes_out"] for r in results], axis=0)
    return vm, ew, eb, sp


def kernel(W, b, Vmem, E_W, E_b, x):
    global _PROG
    if _PROG is None:
        _PROG = build_program()
    nc = _PROG
    in_maps = make_in_maps(W, b, Vmem, E_W, E_b, x)
    res = run_bass_kernel_spmd(nc, in_maps, list(range(NCORES))).results
    return assemble_outputs(res)
